# revision 42
# baseline (speedup 1.0000x reference)
"""Trainium2 Bass kernel for nn_BCNet, Strassen variant.

Stage A uses one level of Strassen over the (j,k) blocks of Wv: 7
products of 48 free-256 matmuls instead of 8 block-products.  W-side
combos are host-precomputed and streamed on the Sync queue through 3
rotating SBUF buffers shared with the stage-B weights; V-side combos
are DVE ops on vt slices; products drain coarsely into bf16 SBUF
accumulators (Scalar copy first, DVE adds after) and per-j-chunk
ReLU+bias produce vact.  PSUM note: matmul start=True resets the WHOLE
bank, so only the even chain of each bank-sharing pair uses start; the
odd chain accumulates from the zero its mate's start left behind.
Stage-C stores ride GpSimd so the Sync queue never blocks on
act-gated stores ahead of the gated combo issues.
"""

import numpy as np

B, NV, NQ = 32, 512, 128
V_DIM, Q_DIM, HK, H_OUT = 2048, 1024, 1536, 8
N_CORES = 8
BPC = B // N_CORES
JC = HK // 128
DCV = V_DIM // 128
DCQ = Q_DIM // 128
NH = NV // 2

_CACHE = {}


def _build_nc():
    import concourse.tile as tile
    from concourse import bacc, mybir
    from contextlib import ExitStack

    bf16 = mybir.dt.bfloat16
    f32 = mybir.dt.float32
    ADD = mybir.AluOpType.add
    SUB = mybir.AluOpType.subtract
    MUL = mybir.AluOpType.mult
    Relu = mybir.ActivationFunctionType.Relu
    Ident = mybir.ActivationFunctionType.Identity
    Copy = mybir.ActivationFunctionType.Copy

    nc = bacc.Bacc()

    vT = nc.declare_dram_parameter("vT", [BPC, V_DIM, NV], bf16, isOutput=False)
    qT = nc.declare_dram_parameter("qT", [Q_DIM, BPC * NQ], bf16, isOutput=False)
    WvT = nc.declare_dram_parameter("WvT", [V_DIM, HK], bf16, isOutput=False)
    cmb = nc.declare_dram_parameter("cmb", [5, 128, DCQ, 768], bf16, isOutput=False)
    WqT = nc.declare_dram_parameter("WqT", [Q_DIM, HK], bf16, isOutput=False)
    bvT = nc.declare_dram_parameter("bvT", [128, JC], f32, isOutput=False)
    bqT = nc.declare_dram_parameter("bqT", [128, JC], f32, isOutput=False)
    hm = nc.declare_dram_parameter("hm", [128, JC, H_OUT], f32, isOutput=False)
    hb = nc.declare_dram_parameter("hb", [128, H_OUT], f32, isOutput=False)
    out = nc.declare_dram_parameter("out", [BPC, H_OUT, NQ, NV], f32, isOutput=True)

    with ExitStack() as ctx:
        tc = ctx.enter_context(tile.TileContext(nc))
        consts = ctx.enter_context(tc.tile_pool(name="consts", bufs=1))
        qpool = ctx.enter_context(tc.tile_pool(name="qpool", bufs=1))
        vin = ctx.enter_context(tc.tile_pool(name="vin", bufs=2))
        vact = ctx.enter_context(tc.tile_pool(name="vact", bufs=2))
        qhp = ctx.enter_context(tc.tile_pool(name="qhp", bufs=1))
        junkp = ctx.enter_context(tc.tile_pool(name="junkp", bufs=1))
        tcomb = ctx.enter_context(tc.tile_pool(name="tcomb", bufs=3))
        csb = ctx.enter_context(tc.tile_pool(name="csb", bufs=4))
        outp = ctx.enter_context(tc.tile_pool(name="outp", bufs=3))
        psM = ctx.enter_context(tc.tile_pool(name="psM", bufs=2, space="PSUM"))
        psC = ctx.enter_context(tc.tile_pool(name="psC", bufs=2, space="PSUM"))

        qt_sb = qpool.tile([128, DCQ, BPC * NQ], bf16)
        wq1_sb = consts.tile([128, DCQ, 1024], bf16, tag="wqbig", bufs=3,
                             name="wq1")
        wq2_sb = consts.tile([128, DCQ, 1024], bf16, tag="wqbig", bufs=3,
                             name="wq2")
        wv11_sb = consts.tile([128, DCQ, 768], bf16, name="wv11")
        wv22_sb = consts.tile([128, DCQ, 768], bf16, name="wv22")
        bq_sb = consts.tile([128, JC], f32)
        bv_sb = consts.tile([128, JC], f32)
        hm_sb = consts.tile([128, JC, H_OUT], f32)
        hb_sb = consts.tile([128, H_OUT], f32)
        vt0_sb = vin.tile([128, DCV, NV], bf16, tag="vt", name="vt0")

        qT_r = qT.rearrange("(d p) n -> p d n", p=128)
        WqT_r = WqT.rearrange("(d p) j -> p d j", p=128)
        WvT_r = WvT.rearrange("(d p) j -> p d j", p=128)
        vT0_r = vT[0].rearrange("(d p) n -> p d n", p=128)

        # warmup junk matmuls (PE p-state ramp) while the first DMAs land
        junk = junkp.tile([128, NV], bf16)
        nc.vector.memset(junk, 0.0)
        ps_junk = psC.tile([128, NV], f32, tag="psC", name="ps_junk")
        for w in range(13):
            nc.tensor.matmul(
                ps_junk[:, 0:256], lhsT=junk[:, 0:128], rhs=junk[:, 0:256],
                start=(w == 0), stop=(w == 12),
            )

        nc.gpsimd.dma_start(out=bq_sb, in_=bqT[:, :])
        nc.gpsimd.dma_start(out=hm_sb, in_=hm[:, :, :])
        nc.gpsimd.dma_start(out=bv_sb, in_=bvT[:, :])
        nc.gpsimd.dma_start(out=hb_sb, in_=hb[:, :])

        # bulk loads: all on the single Sync queue in consumption order
        for d in range(DCQ):
            nc.sync.dma_start(out=qt_sb[:, d, :], in_=qT_r[:, d, :])
            nc.sync.dma_start(out=wq1_sb[:, d, :], in_=WqT_r[:, d, 0:1024])
        for d in range(DCQ):
            nc.sync.dma_start(out=wq2_sb[:, d, 0:512],
                              in_=WqT_r[:, d, 1024:HK])
        for g in range(4):
            nc.sync.dma_start(out=vt0_sb[:, 4 * g:4 * g + 4, :],
                              in_=vT0_r[:, 4 * g:4 * g + 4, :])
        for d in range(DCQ):
            nc.sync.dma_start(out=wv11_sb[:, d, :], in_=WvT_r[:, d, 0:768])
        for d in range(DCQ):
            nc.sync.dma_start(out=wv22_sb[:, d, :],
                              in_=WvT_r[:, DCQ + d, 768:HK])

        # ---- stage B: 8-wide then 4-wide passes ----
        qact_sb = qpool.tile([128, JC, BPC * NQ], bf16)

        def b_chain_targets(npsm, npsc, tagp):
            # psC chains first: they land on the earliest-stopping chain
            # slots, so their freeing activations fire first and the next
            # pass's psC allocation never waits
            targets = []
            for t in range(npsc):
                p = psC.tile([128, NV], f32, tag="psC", name=f"{tagp}c{t}")
                targets.append((p, "p", 0))
            for t in range(npsm):
                m = psM.tile([128, 6, NH], f32, tag="psM", name=f"{tagp}m{t}")
                for c in range(3):
                    targets.append((m, "m", c))
            return targets

        def chain_ap(tgt):
            tile_, kind, c = tgt
            return tile_[:, 2 * c:2 * c + 2, :] if kind == "m" else tile_[:, :]

        def half_ap(tgt, hf):
            tile_, kind, c = tgt
            if kind == "m":
                return tile_[:, 2 * c + hf, :]
            return tile_[:, hf * NH:(hf + 1) * NH]

        def w_slice(j, d):
            if j < 8:
                return wq1_sb[:, d, j * 128:(j + 1) * 128]
            return wq2_sb[:, d, (j - 8) * 128:(j - 8 + 1) * 128]

        def run_b_pass(j0, nj, targets):
            for d in range(DCQ - 2):
                for i in range(nj):
                    nc.tensor.matmul(
                        chain_ap(targets[i]), lhsT=w_slice(j0 + i, d),
                        rhs=qt_sb[:, d, :], start=(d == 0), stop=False,
                    )
            for i in range(nj):
                j = j0 + i
                for d in (DCQ - 2, DCQ - 1):
                    nc.tensor.matmul(
                        chain_ap(targets[i]), lhsT=w_slice(j, d),
                        rhs=qt_sb[:, d, :], start=False, stop=(d == DCQ - 1),
                    )
                for hf in range(2):
                    nc.scalar.activation(
                        out=qact_sb[:, j, hf * NH:(hf + 1) * NH],
                        in_=half_ap(targets[i], hf),
                        func=Relu, bias=bq_sb[:, j:j + 1], scale=1.0,
                    )

        run_b_pass(0, 8, b_chain_targets(2, 2, "psB1"))
        run_b_pass(8, 4, b_chain_targets(1, 1, "psB2"))

        # ---- per-batch Strassen stage A + stage C ----
        vt_tiles = {0: vt0_sb}
        tpre = {}
        for b in range(BPC):
            vt_sb = vt_tiles[b]

            cmb_t = []
            for i in range(5):
                t = consts.tile([128, DCQ, 1024], bf16, tag="wqbig", bufs=3,
                                name=f"cmb{b}_{i}")
                # two d-half DMAs: the d-outer product consumes the first
                # half while the second is still on the wire
                nc.sync.dma_start(out=t[:, 0:4, 0:768], in_=cmb[i][:, 0:4, :])
                nc.sync.dma_start(out=t[:, 4:8, 0:768], in_=cmb[i][:, 4:8, :])
                cmb_t.append(t)
            c3, c1, c5, c2, c4 = cmb_t

            B11 = vt_sb[:, 0:8, 0:NH]
            B12 = vt_sb[:, 0:8, NH:NV]
            B21 = vt_sb[:, 8:16, 0:NH]
            B22 = vt_sb[:, 8:16, NH:NV]

            def tt_tile(name, src0, src1, op):
                t = tcomb.tile([128, DCQ, NH], bf16, tag="tc", name=name)
                nc.vector.tensor_tensor(out=t, in0=src0, in1=src1, op=op)
                return t

            if b == 0:
                t3 = tt_tile("t3_0", B12, B22, SUB)
                t4 = tt_tile("t4_0", B21, B11, SUB)
            else:
                t3, t4 = tpre[b]

            # qh build sits EARLY in the Vector queue (stage C needs it)
            qh_sb = qhp.tile([128, JC, H_OUT * NQ], bf16, tag="qh")
            qa = qact_sb[:, :, b * NQ:(b + 1) * NQ]
            for h in range(H_OUT):
                nc.vector.tensor_tensor(
                    out=qh_sb[:, :, h * NQ:(h + 1) * NQ],
                    in0=qa,
                    in1=hm_sb[:, :, h].broadcast_to([128, JC, NQ]),
                    op=MUL,
                )

            vact_sb = vact.tile([128, JC, NV], bf16, tag="vact")
            c_blocks = {}
            for nm in ("C11", "C12", "C21", "C22"):
                c_blocks[nm] = csb.tile([128, 6, NH], bf16, tag="csb",
                                        name=f"{nm}_{b}")

            V = nc.vector

            def product(lhs_sb, rhs, actions, relu=None, stagger=False):
                pm = psM.tile([128, 6, NH], f32, tag="psM")

                def mm(jc, d):
                    nc.tensor.matmul(
                        pm[:, jc, :],
                        lhsT=lhs_sb[:, d, jc * 128:(jc + 1) * 128],
                        rhs=rhs[:, d, :],
                        start=(d == 0 and jc % 2 == 0),
                        stop=(d == DCQ - 1),
                        skip_group_check=True,
                    )

                def drain(jc_sl, jpos):
                    for dst, op in actions:
                        cb = c_blocks[dst]
                        if op == "copy":
                            nc.scalar.activation(out=cb[:, jc_sl, :],
                                                 in_=pm[:, jc_sl, :],
                                                 func=Copy)
                        else:
                            V.tensor_tensor(out=cb[:, jc_sl, :],
                                            in0=cb[:, jc_sl, :],
                                            in1=pm[:, jc_sl, :], op=op)
                    if relu is not None:
                        nm_, j0_, nsl_ = relu
                        for jc in jpos:
                            j = j0_ + jc
                            nc.scalar.activation(
                                out=vact_sb[:, j, nsl_],
                                in_=c_blocks[nm_][:, jc, :],
                                func=Relu, bias=bv_sb[:, j:j + 1], scale=1.0,
                            )

                if not stagger:
                    for d in range(DCQ):
                        for jc in range(6):
                            mm(jc, d)
                    drain(slice(0, 6), range(6))
                else:
                    for jc in range(6):
                        for d in range(DCQ):
                            mm(jc, d)
                        drain(slice(jc, jc + 1), [jc])

            n1 = slice(0, NH)
            n2 = slice(NH, NV)

            product(wv11_sb, t3, [("C12", "copy"), ("C22", "copy")])     # M3
            product(wv22_sb, t4, [("C11", "copy"), ("C21", "copy")])     # M4
            t1 = tt_tile(f"t1_{b}", B11, B22, ADD)
            product(c3, B22, [("C12", ADD), ("C11", SUB)],
                    relu=("C12", 0, n2))                                 # M5
            t7 = tt_tile(f"t7_{b}", B21, B22, ADD)
            product(c1, t1, [("C11", ADD), ("C22", ADD)])                # M1
            t6 = tt_tile(f"t6_{b}", B11, B12, ADD)
            product(c5, t7, [("C11", ADD)], relu=("C11", 0, n1))         # M7
            product(c2, B11, [("C21", ADD), ("C22", SUB)],
                    relu=("C21", 6, n1))                                 # M2
            # prefetch vt[b+1] and build its t3/t4 now, so they execute on
            # Vector during P6/P7 instead of queueing behind P7's staggered
            # drains (which would stall A(b+1)'s first product)
            if b < BPC - 1:
                nxt = vin.tile([128, DCV, NV], bf16, tag="vt")
                vTn_r = vT[b + 1].rearrange("(d p) n -> p d n", p=128)
                nc.sync.dma_start(out=nxt[:, :, :], in_=vTn_r[:, :, :])
                vt_tiles[b + 1] = nxt
                tpre[b + 1] = (
                    tt_tile(f"t3_{b + 1}", nxt[:, 0:8, NH:NV],
                            nxt[:, 8:16, NH:NV], SUB),
                    tt_tile(f"t4_{b + 1}", nxt[:, 8:16, 0:NH],
                            nxt[:, 0:8, 0:NH], SUB),
                )
            product(c4, t6, [("C22", ADD)], relu=("C22", 6, n2),
                    stagger=True)                                        # M6

            # ---- stage C ----
            for h in range(H_OUT):
                last = (b == BPC - 1 and h == H_OUT - 1)
                if not last:
                    po = psC.tile([128, NV], f32, tag="psC")
                    for j in range(JC):
                        nc.tensor.matmul(
                            po,
                            lhsT=qh_sb[:, j, h * NQ:(h + 1) * NQ],
                            rhs=vact_sb[:, j, :],
                            start=(j == 0), stop=(j == JC - 1),
                        )
                    o_sb = outp.tile([128, NV], f32, tag="osb")
                    nc.scalar.activation(
                        out=o_sb, in_=po,
                        func=Ident, bias=hb_sb[:, h:h + 1], scale=1.0,
                    )
                    nc.gpsimd.dma_start(out=out[b, h, :, :], in_=o_sb)
                else:
                    engs = (nc.gpsimd, nc.sync)
                    for half in range(2):
                        sl = slice(half * NH, (half + 1) * NH)
                        po = psC.tile([128, NH], f32, tag="psC",
                                      name=f"psC_last{half}")
                        for j in range(JC):
                            nc.tensor.matmul(
                                po,
                                lhsT=qh_sb[:, j, h * NQ:(h + 1) * NQ],
                                rhs=vact_sb[:, j, sl],
                                start=(j == 0), stop=(j == JC - 1),
                            )
                        o_sb = outp.tile([128, NH], f32, tag="osb",
                                         name=f"osb_last{half}")
                        nc.scalar.activation(
                            out=o_sb, in_=po,
                            func=Ident, bias=hb_sb[:, h:h + 1], scale=1.0,
                        )
                        engs[half].dma_start(out=out[b, h, :, sl], in_=o_sb)

    nc.compile()
    return nc


def kernel(v, q, Wv, bv, Wq, bq, h_mat, h_bias):
    import ml_dtypes
    from concourse import bass_utils

    bf16 = ml_dtypes.bfloat16

    if "nc" not in _CACHE:
        _CACHE["nc"] = _build_nc()
    nc = _CACHE["nc"]

    v = np.asarray(v, dtype=np.float32)
    q = np.asarray(q, dtype=np.float32)
    Wv = np.asarray(Wv, dtype=np.float32)
    Wq = np.asarray(Wq, dtype=np.float32)
    bv = np.asarray(bv, dtype=np.float32)
    bq = np.asarray(bq, dtype=np.float32)
    h_mat = np.asarray(h_mat, dtype=np.float32)
    h_bias = np.asarray(h_bias, dtype=np.float32)

    vT = np.ascontiguousarray(v.transpose(0, 2, 1)).astype(bf16)
    WvT_f = np.ascontiguousarray(Wv.T)
    WvT = WvT_f.astype(bf16)
    WqT = np.ascontiguousarray(Wq.T).astype(bf16)
    bvT = np.ascontiguousarray(bv.reshape(JC, 128).T)
    bqT = np.ascontiguousarray(bq.reshape(JC, 128).T)
    hmP = np.ascontiguousarray(h_mat.reshape(H_OUT, JC, 128).transpose(2, 1, 0))
    hbB = np.ascontiguousarray(np.broadcast_to(h_bias[None, :], (128, H_OUT)))

    T = WvT_f
    combos = [
        T[0:1024, 0:768] + T[1024:2048, 0:768],        # A11+A12 (c3, M5)
        T[0:1024, 0:768] + T[1024:2048, 768:1536],     # A11+A22 (c1, M1)
        T[1024:2048, 0:768] - T[1024:2048, 768:1536],  # A12-A22 (c5, M7)
        T[0:1024, 768:1536] + T[1024:2048, 768:1536],  # A21+A22 (c2, M2)
        T[0:1024, 768:1536] - T[0:1024, 0:768],        # A21-A11 (c4, M6)
    ]
    cmbA = np.stack([
        np.ascontiguousarray(
            c.reshape(DCQ, 128, 768).transpose(1, 0, 2)).astype(bf16)
        for c in combos
    ])

    in_maps = []
    for c in range(N_CORES):
        bs = slice(BPC * c, BPC * (c + 1))
        qTc = np.ascontiguousarray(
            q[bs].transpose(2, 0, 1).reshape(Q_DIM, BPC * NQ)
        ).astype(bf16)
        in_maps.append({
            "vT": vT[bs],
            "qT": qTc,
            "WvT": WvT,
            "cmb": cmbA,
            "WqT": WqT,
            "bvT": bvT,
            "bqT": bqT,
            "hm": hmP,
            "hb": hbB,
        })

    res = bass_utils.run_bass_kernel_spmd(nc, in_maps, list(range(N_CORES)))
    outs = np.concatenate([res.results[c]["out"] for c in range(N_CORES)], axis=0)
    logits = outs.transpose(0, 1, 3, 2)
    return np.ascontiguousarray(logits)


# revision 43
# speedup vs baseline: 1.0081x; 1.0081x over previous
"""Trainium2 Bass kernel for nn_BCNet, Strassen variant.

Stage A uses one level of Strassen over the (j,k) blocks of Wv: 7
products of 48 free-256 matmuls instead of 8 block-products.  W-side
combos are host-precomputed and streamed on the Sync queue through 3
rotating SBUF buffers shared with the stage-B weights; V-side combos
are DVE ops on vt slices; products drain coarsely into bf16 SBUF
accumulators (Scalar copy first, DVE adds after) and per-j-chunk
ReLU+bias produce vact.  PSUM note: matmul start=True resets the WHOLE
bank, so only the even chain of each bank-sharing pair uses start; the
odd chain accumulates from the zero its mate's start left behind.
Stage-C stores ride GpSimd so the Sync queue never blocks on
act-gated stores ahead of the gated combo issues.
"""

import numpy as np

B, NV, NQ = 32, 512, 128
V_DIM, Q_DIM, HK, H_OUT = 2048, 1024, 1536, 8
N_CORES = 8
BPC = B // N_CORES
JC = HK // 128
DCV = V_DIM // 128
DCQ = Q_DIM // 128
NH = NV // 2

_CACHE = {}


def _build_nc():
    import concourse.tile as tile
    from concourse import bacc, mybir
    from contextlib import ExitStack

    bf16 = mybir.dt.bfloat16
    f32 = mybir.dt.float32
    ADD = mybir.AluOpType.add
    SUB = mybir.AluOpType.subtract
    MUL = mybir.AluOpType.mult
    Relu = mybir.ActivationFunctionType.Relu
    Ident = mybir.ActivationFunctionType.Identity
    Copy = mybir.ActivationFunctionType.Copy

    nc = bacc.Bacc()

    vT = nc.declare_dram_parameter("vT", [BPC, V_DIM, NV], bf16, isOutput=False)
    qT = nc.declare_dram_parameter("qT", [Q_DIM, BPC * NQ], bf16, isOutput=False)
    WvT = nc.declare_dram_parameter("WvT", [V_DIM, HK], bf16, isOutput=False)
    cmb = nc.declare_dram_parameter("cmb", [5, 128, DCQ, 768], bf16, isOutput=False)
    WqT = nc.declare_dram_parameter("WqT", [Q_DIM, HK], bf16, isOutput=False)
    bvT = nc.declare_dram_parameter("bvT", [128, JC], f32, isOutput=False)
    bqT = nc.declare_dram_parameter("bqT", [128, JC], f32, isOutput=False)
    hm = nc.declare_dram_parameter("hm", [128, JC, H_OUT], f32, isOutput=False)
    hb = nc.declare_dram_parameter("hb", [128, H_OUT], f32, isOutput=False)
    out = nc.declare_dram_parameter("out", [BPC, H_OUT, NQ, NV], f32, isOutput=True)

    with ExitStack() as ctx:
        tc = ctx.enter_context(tile.TileContext(nc))
        consts = ctx.enter_context(tc.tile_pool(name="consts", bufs=1))
        qpool = ctx.enter_context(tc.tile_pool(name="qpool", bufs=1))
        vin = ctx.enter_context(tc.tile_pool(name="vin", bufs=2))
        vact = ctx.enter_context(tc.tile_pool(name="vact", bufs=2))
        qhp = ctx.enter_context(tc.tile_pool(name="qhp", bufs=1))
        junkp = ctx.enter_context(tc.tile_pool(name="junkp", bufs=1))
        tcomb = ctx.enter_context(tc.tile_pool(name="tcomb", bufs=3))
        csb = ctx.enter_context(tc.tile_pool(name="csb", bufs=4))
        outp = ctx.enter_context(tc.tile_pool(name="outp", bufs=3))
        psM = ctx.enter_context(tc.tile_pool(name="psM", bufs=2, space="PSUM"))
        psC = ctx.enter_context(tc.tile_pool(name="psC", bufs=2, space="PSUM"))

        qt_sb = qpool.tile([128, DCQ, BPC * NQ], bf16)
        wq1_sb = consts.tile([128, DCQ, 1024], bf16, tag="wqbig", bufs=3,
                             name="wq1")
        wq2_sb = consts.tile([128, DCQ, 1024], bf16, tag="wqbig", bufs=3,
                             name="wq2")
        wv11_sb = consts.tile([128, DCQ, 768], bf16, name="wv11")
        wv22_sb = consts.tile([128, DCQ, 768], bf16, name="wv22")
        bq_sb = consts.tile([128, JC], f32)
        bv_sb = consts.tile([128, JC], f32)
        hm_sb = consts.tile([128, JC, H_OUT], f32)
        hb_sb = consts.tile([128, H_OUT], f32)
        vt0_sb = vin.tile([128, DCV, NV], bf16, tag="vt", name="vt0")

        qT_r = qT.rearrange("(d p) n -> p d n", p=128)
        WqT_r = WqT.rearrange("(d p) j -> p d j", p=128)
        WvT_r = WvT.rearrange("(d p) j -> p d j", p=128)
        vT0_r = vT[0].rearrange("(d p) n -> p d n", p=128)

        # warmup junk matmuls (PE p-state ramp) while the first DMAs land
        junk = junkp.tile([128, NV], bf16)
        nc.vector.memset(junk, 0.0)
        ps_junk = psC.tile([128, NV], f32, tag="psC", name="ps_junk")
        for w in range(13):
            nc.tensor.matmul(
                ps_junk[:, 0:256], lhsT=junk[:, 0:128], rhs=junk[:, 0:256],
                start=(w == 0), stop=(w == 12),
            )

        nc.gpsimd.dma_start(out=bq_sb, in_=bqT[:, :])
        nc.gpsimd.dma_start(out=hm_sb, in_=hm[:, :, :])
        nc.gpsimd.dma_start(out=bv_sb, in_=bvT[:, :])
        nc.gpsimd.dma_start(out=hb_sb, in_=hb[:, :])

        # bulk loads: all on the single Sync queue in consumption order
        for d in range(DCQ):
            nc.sync.dma_start(out=qt_sb[:, d, :], in_=qT_r[:, d, :])
            nc.sync.dma_start(out=wq1_sb[:, d, :], in_=WqT_r[:, d, 0:1024])
        for d in range(DCQ):
            nc.sync.dma_start(out=wq2_sb[:, d, 0:512],
                              in_=WqT_r[:, d, 1024:HK])
        for g in range(4):
            nc.sync.dma_start(out=vt0_sb[:, 4 * g:4 * g + 4, :],
                              in_=vT0_r[:, 4 * g:4 * g + 4, :])
        for d in range(DCQ):
            nc.sync.dma_start(out=wv11_sb[:, d, :], in_=WvT_r[:, d, 0:768])
        for d in range(DCQ):
            nc.sync.dma_start(out=wv22_sb[:, d, :],
                              in_=WvT_r[:, DCQ + d, 768:HK])

        # ---- stage B: 8-wide then 4-wide passes ----
        qact_sb = qpool.tile([128, JC, BPC * NQ], bf16)

        def b_chain_targets(npsm, npsc, tagp):
            targets = []
            for t in range(npsm):
                m = psM.tile([128, 6, NH], f32, tag="psM", name=f"{tagp}m{t}")
                for c in range(3):
                    targets.append((m, "m", c))
            for t in range(npsc):
                p = psC.tile([128, NV], f32, tag="psC", name=f"{tagp}c{t}")
                targets.append((p, "p", 0))
            return targets

        def chain_ap(tgt):
            tile_, kind, c = tgt
            return tile_[:, 2 * c:2 * c + 2, :] if kind == "m" else tile_[:, :]

        def half_ap(tgt, hf):
            tile_, kind, c = tgt
            if kind == "m":
                return tile_[:, 2 * c + hf, :]
            return tile_[:, hf * NH:(hf + 1) * NH]

        def w_slice(j, d):
            if j < 8:
                return wq1_sb[:, d, j * 128:(j + 1) * 128]
            return wq2_sb[:, d, (j - 8) * 128:(j - 8 + 1) * 128]

        def run_b_pass(j0, nj, targets):
            for d in range(DCQ - 2):
                for i in range(nj):
                    nc.tensor.matmul(
                        chain_ap(targets[i]), lhsT=w_slice(j0 + i, d),
                        rhs=qt_sb[:, d, :], start=(d == 0), stop=False,
                    )
            for i in range(nj):
                j = j0 + i
                for d in (DCQ - 2, DCQ - 1):
                    nc.tensor.matmul(
                        chain_ap(targets[i]), lhsT=w_slice(j, d),
                        rhs=qt_sb[:, d, :], start=False, stop=(d == DCQ - 1),
                    )
                for hf in range(2):
                    nc.scalar.activation(
                        out=qact_sb[:, j, hf * NH:(hf + 1) * NH],
                        in_=half_ap(targets[i], hf),
                        func=Relu, bias=bq_sb[:, j:j + 1], scale=1.0,
                    )

        run_b_pass(0, 8, b_chain_targets(2, 2, "psB1"))
        run_b_pass(8, 4, b_chain_targets(1, 1, "psB2"))

        # ---- per-batch Strassen stage A + stage C ----
        vt_tiles = {0: vt0_sb}
        tpre = {}
        for b in range(BPC):
            vt_sb = vt_tiles[b]

            cmb_t = []
            for i in range(5):
                t = consts.tile([128, DCQ, 1024], bf16, tag="wqbig", bufs=3,
                                name=f"cmb{b}_{i}")
                # two d-half DMAs: the d-outer product consumes the first
                # half while the second is still on the wire
                nc.sync.dma_start(out=t[:, 0:4, 0:768], in_=cmb[i][:, 0:4, :])
                nc.sync.dma_start(out=t[:, 4:8, 0:768], in_=cmb[i][:, 4:8, :])
                cmb_t.append(t)
            c3, c1, c5, c2, c4 = cmb_t

            B11 = vt_sb[:, 0:8, 0:NH]
            B12 = vt_sb[:, 0:8, NH:NV]
            B21 = vt_sb[:, 8:16, 0:NH]
            B22 = vt_sb[:, 8:16, NH:NV]

            def tt_tile(name, src0, src1, op):
                t = tcomb.tile([128, DCQ, NH], bf16, tag="tc", name=name)
                nc.vector.tensor_tensor(out=t, in0=src0, in1=src1, op=op)
                return t

            if b == 0:
                t3 = tt_tile("t3_0", B12, B22, SUB)
                t4 = tt_tile("t4_0", B21, B11, SUB)
            else:
                t3, t4 = tpre[b]

            # qh build sits EARLY in the Vector queue (stage C needs it)
            qh_sb = qhp.tile([128, JC, H_OUT * NQ], bf16, tag="qh")
            qa = qact_sb[:, :, b * NQ:(b + 1) * NQ]
            for h in range(H_OUT):
                nc.vector.tensor_tensor(
                    out=qh_sb[:, :, h * NQ:(h + 1) * NQ],
                    in0=qa,
                    in1=hm_sb[:, :, h].broadcast_to([128, JC, NQ]),
                    op=MUL,
                )

            vact_sb = vact.tile([128, JC, NV], bf16, tag="vact")
            c_blocks = {}
            for nm in ("C11", "C12", "C21", "C22"):
                c_blocks[nm] = csb.tile([128, 6, NH], bf16, tag="csb",
                                        name=f"{nm}_{b}")

            V = nc.vector

            def product(lhs_sb, rhs, actions, relu=None, stagger=False):
                pm = psM.tile([128, 6, NH], f32, tag="psM")

                def mm(jc, d):
                    nc.tensor.matmul(
                        pm[:, jc, :],
                        lhsT=lhs_sb[:, d, jc * 128:(jc + 1) * 128],
                        rhs=rhs[:, d, :],
                        start=(d == 0 and jc % 2 == 0),
                        stop=(d == DCQ - 1),
                        skip_group_check=True,
                    )

                def drain(jc_sl, jpos):
                    for dst, op in actions:
                        cb = c_blocks[dst]
                        if op == "copy":
                            nc.scalar.activation(out=cb[:, jc_sl, :],
                                                 in_=pm[:, jc_sl, :],
                                                 func=Copy)
                        else:
                            V.tensor_tensor(out=cb[:, jc_sl, :],
                                            in0=cb[:, jc_sl, :],
                                            in1=pm[:, jc_sl, :], op=op)
                    if relu is not None:
                        nm_, j0_, nsl_ = relu
                        for jc in jpos:
                            j = j0_ + jc
                            nc.scalar.activation(
                                out=vact_sb[:, j, nsl_],
                                in_=c_blocks[nm_][:, jc, :],
                                func=Relu, bias=bv_sb[:, j:j + 1], scale=1.0,
                            )

                if not stagger:
                    for d in range(DCQ):
                        for jc in range(6):
                            mm(jc, d)
                    drain(slice(0, 6), range(6))
                else:
                    for jc in range(6):
                        for d in range(DCQ):
                            mm(jc, d)
                        drain(slice(jc, jc + 1), [jc])

            n1 = slice(0, NH)
            n2 = slice(NH, NV)

            product(wv11_sb, t3, [("C12", "copy"), ("C22", "copy")])     # M3
            product(wv22_sb, t4, [("C11", "copy"), ("C21", "copy")])     # M4
            t1 = tt_tile(f"t1_{b}", B11, B22, ADD)
            product(c3, B22, [("C12", ADD), ("C11", SUB)],
                    relu=("C12", 0, n2))                                 # M5
            t7 = tt_tile(f"t7_{b}", B21, B22, ADD)
            product(c1, t1, [("C11", ADD), ("C22", ADD)])                # M1
            t6 = tt_tile(f"t6_{b}", B11, B12, ADD)
            product(c5, t7, [("C11", ADD)], relu=("C11", 0, n1))         # M7
            product(c2, B11, [("C21", ADD), ("C22", SUB)],
                    relu=("C21", 6, n1))                                 # M2
            # prefetch vt[b+1] and build its t3/t4 now, so they execute on
            # Vector during P6/P7 instead of queueing behind P7's staggered
            # drains (which would stall A(b+1)'s first product)
            if b < BPC - 1:
                nxt = vin.tile([128, DCV, NV], bf16, tag="vt")
                vTn_r = vT[b + 1].rearrange("(d p) n -> p d n", p=128)
                nc.sync.dma_start(out=nxt[:, :, :], in_=vTn_r[:, :, :])
                vt_tiles[b + 1] = nxt
                tpre[b + 1] = (
                    tt_tile(f"t3_{b + 1}", nxt[:, 0:8, NH:NV],
                            nxt[:, 8:16, NH:NV], SUB),
                    tt_tile(f"t4_{b + 1}", nxt[:, 8:16, 0:NH],
                            nxt[:, 0:8, 0:NH], SUB),
                )
            product(c4, t6, [("C22", ADD)], relu=("C22", 6, n2),
                    stagger=True)                                        # M6

            # ---- stage C ----
            for h in range(H_OUT):
                last = (b == BPC - 1 and h == H_OUT - 1)
                if not last:
                    po = psC.tile([128, NV], f32, tag="psC")
                    for j in range(JC):
                        nc.tensor.matmul(
                            po,
                            lhsT=qh_sb[:, j, h * NQ:(h + 1) * NQ],
                            rhs=vact_sb[:, j, :],
                            start=(j == 0), stop=(j == JC - 1),
                        )
                    o_sb = outp.tile([128, NV], f32, tag="osb")
                    nc.scalar.activation(
                        out=o_sb, in_=po,
                        func=Ident, bias=hb_sb[:, h:h + 1], scale=1.0,
                    )
                    nc.gpsimd.dma_start(out=out[b, h, :, :], in_=o_sb)
                else:
                    engs = (nc.gpsimd, nc.sync)
                    for half in range(2):
                        sl = slice(half * NH, (half + 1) * NH)
                        po = psC.tile([128, NH], f32, tag="psC",
                                      name=f"psC_last{half}")
                        for j in range(JC):
                            nc.tensor.matmul(
                                po,
                                lhsT=qh_sb[:, j, h * NQ:(h + 1) * NQ],
                                rhs=vact_sb[:, j, sl],
                                start=(j == 0), stop=(j == JC - 1),
                            )
                        o_sb = outp.tile([128, NH], f32, tag="osb",
                                         name=f"osb_last{half}")
                        nc.scalar.activation(
                            out=o_sb, in_=po,
                            func=Ident, bias=hb_sb[:, h:h + 1], scale=1.0,
                        )
                        engs[half].dma_start(out=out[b, h, :, sl], in_=o_sb)

    nc.compile()
    return nc


def kernel(v, q, Wv, bv, Wq, bq, h_mat, h_bias):
    import ml_dtypes
    from concourse import bass_utils

    bf16 = ml_dtypes.bfloat16

    if "nc" not in _CACHE:
        _CACHE["nc"] = _build_nc()
    nc = _CACHE["nc"]

    v = np.asarray(v, dtype=np.float32)
    q = np.asarray(q, dtype=np.float32)
    Wv = np.asarray(Wv, dtype=np.float32)
    Wq = np.asarray(Wq, dtype=np.float32)
    bv = np.asarray(bv, dtype=np.float32)
    bq = np.asarray(bq, dtype=np.float32)
    h_mat = np.asarray(h_mat, dtype=np.float32)
    h_bias = np.asarray(h_bias, dtype=np.float32)

    vT = np.ascontiguousarray(v.transpose(0, 2, 1)).astype(bf16)
    WvT_f = np.ascontiguousarray(Wv.T)
    WvT = WvT_f.astype(bf16)
    WqT = np.ascontiguousarray(Wq.T).astype(bf16)
    bvT = np.ascontiguousarray(bv.reshape(JC, 128).T)
    bqT = np.ascontiguousarray(bq.reshape(JC, 128).T)
    hmP = np.ascontiguousarray(h_mat.reshape(H_OUT, JC, 128).transpose(2, 1, 0))
    hbB = np.ascontiguousarray(np.broadcast_to(h_bias[None, :], (128, H_OUT)))

    T = WvT_f
    combos = [
        T[0:1024, 0:768] + T[1024:2048, 0:768],        # A11+A12 (c3, M5)
        T[0:1024, 0:768] + T[1024:2048, 768:1536],     # A11+A22 (c1, M1)
        T[1024:2048, 0:768] - T[1024:2048, 768:1536],  # A12-A22 (c5, M7)
        T[0:1024, 768:1536] + T[1024:2048, 768:1536],  # A21+A22 (c2, M2)
        T[0:1024, 768:1536] - T[0:1024, 0:768],        # A21-A11 (c4, M6)
    ]
    cmbA = np.stack([
        np.ascontiguousarray(
            c.reshape(DCQ, 128, 768).transpose(1, 0, 2)).astype(bf16)
        for c in combos
    ])

    in_maps = []
    for c in range(N_CORES):
        bs = slice(BPC * c, BPC * (c + 1))
        qTc = np.ascontiguousarray(
            q[bs].transpose(2, 0, 1).reshape(Q_DIM, BPC * NQ)
        ).astype(bf16)
        in_maps.append({
            "vT": vT[bs],
            "qT": qTc,
            "WvT": WvT,
            "cmb": cmbA,
            "WqT": WqT,
            "bvT": bvT,
            "bqT": bqT,
            "hm": hmP,
            "hb": hbB,
        })

    res = bass_utils.run_bass_kernel_spmd(nc, in_maps, list(range(N_CORES)))
    outs = np.concatenate([res.results[c]["out"] for c in range(N_CORES)], axis=0)
    logits = outs.transpose(0, 1, 3, 2)
    return np.ascontiguousarray(logits)
